# revision 1
# baseline (speedup 1.0000x reference)
"""Trainium2 Bass kernel for the DLGN kernel-machine problem.

Reference computation (fp32):
    ig = inp @ gating[0]; dg = data @ gating[0]
    K  = sig(B*ig) @ sig(B*dg).T
    for l in 1..3:
        ig = ig @ gating[l]; dg = dg @ gating[l]
        K *= (sig(B*ig) @ sig(B*dg).T) / 512
    out = K @ alphas                      # [n_inp]

Shapes: inp [4096, 512], data [8192, 512], gating [4, 512, 512],
alphas [8192]; out [4096] fp32.

Strategy (8 NeuronCores):
  - 2D shard: inp rows into R=4 groups of 1024 (replicated over C), data rows
    into C=2 groups of 4096 (replicated over R). core = r*C + c. Each core
    computes y_partial[r-block] = K_block @ alphas[c-block]; host sums the C
    partials and concatenates the R blocks. No on-device collectives.
  - All matmuls run as float32r (TF32-like, ~1.3e-4 rel err, full PE rate at
    N>=256) with the contraction dim on SBUF partitions. The host feeds the
    activations pre-transposed ([512, n]) so the gate chain
    igT_l = W_l^T-contract(igT_{l-1}) stays in transposed layout with zero
    on-device transposes.
  - Per core: phase A computes the i-side gate chain for all 4 layers
    (sig_i resident, 64KB/partition). Then 8 d-stripes of 512: d-side gate
    chain for the stripe, alphas folded into sig_d layer 3, then the K-product
    matmuls with the running elementwise product kept on the vector engine
    (scalar_tensor_tensor with accum_out does the final mult+row-sum in one
    pass; tensor_tensor_reduce is avoided -- it hangs TRN2 under this runtime).
  - ACT engine runs ONLY Sigmoid ops (PSUM->SBUF pre-activation copies live on
    the vector engine instead): mixing Copy and Sigmoid on ACT thrashes the
    activation table (~9x per-op penalty, measured +230us end-to-end).
  - dataT stripe DMAs are split per k-chunk across queues and dat/PSUM pools
    are triple-buffered: measured -45us/iter (DMA was on the critical path).
  - sig_bf16=True variant stores sigs in bf16 (K matmuls bf16, sig_d double-
    buffered): ~8% faster, rel err 8.7e-4 vs 1.6e-4; f32r is the default.
"""

import numpy as np

import concourse.tile as tile
from concourse import bacc, mybir
from concourse.bass_utils import run_bass_kernel_spmd

BETA = 4.0
WIDTH = 512
DEPTH = 4
DIM = 512
N_I = 4096
N_D = 8192
R, C = 4, 2
NI_SH = N_I // R  # 1024
ND_SH = N_D // C  # 4096
D_STRIPE = 512
N_STRIPES = ND_SH // D_STRIPE  # 8
I_CHUNKS = NI_SH // 128  # 8
KC = DIM // 128  # 4 contraction chunks

F32 = mybir.dt.float32
F32R = mybir.dt.float32r
BF16 = mybir.dt.bfloat16
AFT = mybir.ActivationFunctionType
MULT = mybir.AluOpType.mult
ADD = mybir.AluOpType.add

_NC = None


def _build(repeat=1, sig_bf16=False):
    nc = bacc.Bacc("TRN2", target_bir_lowering=False, debug=False, num_devices=8)

    inpT_d = nc.dram_tensor("inpT", [DIM, NI_SH], F32R, kind="ExternalInput")
    dataT_d = nc.dram_tensor("dataT", [DIM, ND_SH], F32R, kind="ExternalInput")
    gating_d = nc.dram_tensor("gating", [DEPTH, DIM, DIM], F32R, kind="ExternalInput")
    alphas_d = nc.dram_tensor("alphas_b", [128, ND_SH], F32, kind="ExternalInput")
    y_d = nc.dram_tensor("y", [128, I_CHUNKS], F32, kind="ExternalOutput")

    SIG_DT = BF16 if sig_bf16 else F32R
    from contextlib import nullcontext

    with tile.TileContext(nc) as tc:
        with (
            tc.tile_pool(name="w", bufs=1) as wpool,
            tc.tile_pool(name="sigi", bufs=1) as sigi_pool,
            tc.tile_pool(name="yp", bufs=1) as ypool,
            tc.tile_pool(name="gpsum", bufs=3, space="PSUM") as gpsum,
            tc.tile_pool(name="kpsum", bufs=3, space="PSUM") as kpsum,
            tc.For_i(0, repeat, 1) if repeat > 1 else nullcontext(),
        ):
            W = wpool.tile([128, DEPTH, KC, DIM], F32R)
            for l in range(DEPTH):
                nc.sync.dma_start(
                    W[:, l],
                    gating_d.ap()[l].rearrange("(k p) n -> p k n", p=128),
                )

            sig_i = sigi_pool.tile([128, DEPTH, KC, NI_SH], SIG_DT)
            y_acc = ypool.tile([128, I_CHUNKS], F32)
            nc.gpsimd.memset(y_acc[:], 0.0)

            # ---- Phase A: i-side gate chain, all layers ----
            with tc.tile_pool(name="ig", bufs=2) as igpool:
                prev = igpool.tile([128, KC, NI_SH], F32R, tag="ig")
                inpT_r = inpT_d.ap().rearrange("(k p) n -> p k n", p=128)
                for k in range(KC):
                    nc.sync.dma_start(prev[:, k], inpT_r[:, k])
                for l in range(DEPTH):
                    nxt = (
                        igpool.tile([128, KC, NI_SH], F32R, tag="ig", name=f"ig{l}")
                        if l < DEPTH - 1
                        else None
                    )
                    for m in range(KC):
                        for nb in range(NI_SH // 512):
                            sl = slice(nb * 512, (nb + 1) * 512)
                            ps = gpsum.tile([128, 512], F32, tag="gps")
                            for k in range(KC):
                                nc.tensor.matmul(
                                    ps[:],
                                    W[:, l, k, m * 128 : (m + 1) * 128],
                                    prev[:, k, sl],
                                    start=(k == 0),
                                    stop=(k == KC - 1),
                                )
                            nc.scalar.activation(
                                sig_i[:, l, m, sl], ps[:], AFT.Sigmoid, scale=BETA
                            )
                            if nxt is not None:
                                nc.vector.tensor_copy(nxt[:, m, sl], ps[:])
                    prev = nxt

            # ---- Phase B: d-stripes ----
            with (
                tc.tile_pool(name="dat", bufs=3) as datpool,
                tc.tile_pool(name="dg", bufs=2) as dgpool,
                tc.tile_pool(name="sigd", bufs=2 if sig_bf16 else 1) as sigd_pool,
                tc.tile_pool(name="alp", bufs=2) as alpool,
                tc.tile_pool(name="kblk", bufs=2) as kpool,
                tc.tile_pool(name="scr", bufs=2) as scrpool,
            ):
                for s in range(N_STRIPES):
                    ssl = slice(s * D_STRIPE, (s + 1) * D_STRIPE)
                    dat = datpool.tile([128, KC, D_STRIPE], F32R, tag="dat")
                    dat_r = dataT_d.ap()[:, ssl].rearrange("(k p) n -> p k n", p=128)
                    for k in range(KC):
                        nc.sync.dma_start(dat[:, k], dat_r[:, k])
                    alp = alpool.tile([128, D_STRIPE], F32, tag="alp")
                    nc.sync.dma_start(alp[:], alphas_d.ap()[:, ssl])

                    sig_d = sigd_pool.tile([128, DEPTH, KC, D_STRIPE], SIG_DT)

                    # d-side gate chain for this stripe
                    prev = dat
                    for l in range(DEPTH):
                        nxt = (
                            dgpool.tile(
                                [128, KC, D_STRIPE], F32R, tag="dg", name=f"dg{s}_{l}"
                            )
                            if l < DEPTH - 1
                            else None
                        )
                        for m in range(KC):
                            ps = gpsum.tile([128, 512], F32, tag="gps")
                            for k in range(KC):
                                nc.tensor.matmul(
                                    ps[:],
                                    W[:, l, k, m * 128 : (m + 1) * 128],
                                    prev[:, k, :],
                                    start=(k == 0),
                                    stop=(k == KC - 1),
                                )
                            nc.scalar.activation(
                                sig_d[:, l, m, :], ps[:], AFT.Sigmoid, scale=BETA
                            )
                            if nxt is not None:
                                nc.vector.tensor_copy(nxt[:, m, :], ps[:])
                            else:
                                # layer 3: fold alphas into sig_d
                                nc.vector.tensor_mul(
                                    sig_d[:, 3, m, :], sig_d[:, 3, m, :], alp[:]
                                )
                        prev = nxt

                    # K-product matmuls + running product + y reduce
                    for ic in range(I_CHUNKS):
                        isl = slice(ic * 128, (ic + 1) * 128)
                        kblk = kpool.tile([128, D_STRIPE], F32, tag="kblk")
                        for l in range(DEPTH):
                            ps = kpsum.tile([128, 512], F32, tag="kps")
                            for k in range(KC):
                                nc.tensor.matmul(
                                    ps[:],
                                    sig_i[:, l, k, isl],
                                    sig_d[:, l, k, :],
                                    start=(k == 0),
                                    stop=(k == KC - 1),
                                )
                            if l == 0:
                                nc.vector.tensor_copy(kblk[:], ps[:])
                            elif l < DEPTH - 1:
                                # kblk = (ps * 1/512) * kblk
                                nc.vector.scalar_tensor_tensor(
                                    kblk[:], ps[:], 1.0 / WIDTH, kblk[:], MULT, MULT
                                )
                            else:
                                # y[:, ic] += sum_d (ps * 1/512) * kblk
                                part = scrpool.tile([128, 1], F32, tag="part")
                                nc.vector.scalar_tensor_tensor(
                                    kblk[:],
                                    ps[:],
                                    1.0 / WIDTH,
                                    kblk[:],
                                    MULT,
                                    MULT,
                                    accum_out=part[:, 0:1],
                                )
                                nc.vector.tensor_add(
                                    y_acc[:, ic : ic + 1],
                                    y_acc[:, ic : ic + 1],
                                    part[:, 0:1],
                                )

            nc.sync.dma_start(y_d.ap(), y_acc[:])

    nc.compile()
    return nc


def _get_nc():
    global _NC
    if _NC is None:
        _NC = _build()
    return _NC


def kernel(inp, data, gating, alphas):
    inp = np.ascontiguousarray(np.asarray(inp, dtype=np.float32))
    data = np.ascontiguousarray(np.asarray(data, dtype=np.float32))
    gating = np.ascontiguousarray(np.asarray(gating, dtype=np.float32))
    alphas = np.ascontiguousarray(np.asarray(alphas, dtype=np.float32))

    nc = _get_nc()

    in_maps = []
    for r in range(R):
        inpT = np.ascontiguousarray(inp[r * NI_SH : (r + 1) * NI_SH].T)
        for c in range(C):
            dataT = np.ascontiguousarray(data[c * ND_SH : (c + 1) * ND_SH].T)
            al = np.ascontiguousarray(
                np.broadcast_to(alphas[c * ND_SH : (c + 1) * ND_SH], (128, ND_SH))
            )
            in_maps.append(
                {"inpT": inpT, "dataT": dataT, "gating": gating, "alphas_b": al}
            )

    res = run_bass_kernel_spmd(nc, in_maps, core_ids=list(range(R * C))).results

    y = np.empty(N_I, dtype=np.float32)
    for r in range(R):
        acc = res[r * C]["y"].T.reshape(NI_SH).copy()
        for c in range(1, C):
            acc += res[r * C + c]["y"].T.reshape(NI_SH)
        y[r * NI_SH : (r + 1) * NI_SH] = acc
    return y



# revision 2
# speedup vs baseline: 1.1009x; 1.1009x over previous
"""Trainium2 Bass kernel for the DLGN kernel-machine problem (fp8 DoubleRow).

Reference (fp32):
    ig = inp @ g0; dg = data @ g0
    K  = sig(4 ig) @ sig(4 dg).T
    for l in 1..3: ig @= g_l; dg @= g_l; K *= (sig(4 ig) @ sig(4 dg).T)/512
    out = K @ alphas                                  # [4096]

Design (8 cores, R=2 x C=4: inp rows split in 2, data rows in 4; each core
computes y_partial[r-block] over its data block; host sums C partials):
  - Gate chains are LINEAR in the preactivations: ig_l = inp @ (g0 g1..g_l).
    The kernel precomputes both weight-product chains on device (WP_l and
    its transpose chain V_l, plain f32r matmuls) and computes every layer's
    preactivation directly from inp/data -- no serial layer dependency, no
    PSUM->SBUF preactivation copies.
  - K-product runs in fp8 (float8e4) DoubleRow mode at 0.5 cyc/row (4x the
    f32r rate). Precision is recovered by tanh-centering: store
    t = fp8(tanh(2x)) (= 2 sig(4x) - 1), then
      sig-gram G = (T + u_i + u_d + 512) / 4,  T = t_i^T t_d (fp8 gram),
    with u_* = exact column sums of t (ones-matmuls). u_i enters as a
    per-partition scalar in the vector-engine multiply; u_d via one extra
    zero-padded DoubleRow pair (row0 = 64 * fp8(u_d/64)). The constant
    1/(4*2048^3) is applied once to y at the end (2^-35). Measured rel err
    ~1e-2 vs the 2e-2 gate (CPU model 9.8e-3).
  - ACT engine runs ONLY Tanh (scale=2, direct fp8 output -- bit-exact vs
    RNE cast); running K-product stays on DVE scalar_tensor_tensor with the
    alphas folded into layer 0 and accum_out row-sum on layer 3.
"""

import numpy as np
import ml_dtypes

import concourse.tile as tile
from concourse import bacc, mybir
from concourse.bass_utils import run_bass_kernel_spmd

WIDTH = 512
DEPTH = 4
DIM = 512
N_I = 4096
N_D = 8192
R, C = 2, 4
NI_SH = N_I // R  # 2048
ND_SH = N_D // C  # 2048
I_CHUNKS = NI_SH // 128  # 16
DSL = ND_SH // 512  # 4 d-blocks of 512
KC = DIM // 128  # 4

F32 = mybir.dt.float32
F32R = mybir.dt.float32r
FP8 = mybir.dt.float8e4
AFT = mybir.ActivationFunctionType
MULT = mybir.AluOpType.mult
ADD = mybir.AluOpType.add
DR = mybir.MatmulPerfMode.DoubleRow

_NC = None


def _build_fp8(repeat=1):
    nc = bacc.Bacc("TRN2", target_bir_lowering=False, debug=False, num_devices=8)

    inpT_d = nc.dram_tensor("inpT", [DIM, NI_SH], F32R, kind="ExternalInput")
    dataT_d = nc.dram_tensor("dataT", [DIM, ND_SH], F32R, kind="ExternalInput")
    gating_d = nc.dram_tensor("gating", [DEPTH, DIM, DIM], F32R, kind="ExternalInput")
    g0T_d = nc.dram_tensor("g0T", [DIM, DIM], F32R, kind="ExternalInput")
    alphas_d = nc.dram_tensor("alphas_b", [128, ND_SH], F32, kind="ExternalInput")
    ones_d = nc.dram_tensor("ones8", [128, 2, 16], FP8, kind="ExternalInput")
    corrl_d = nc.dram_tensor("corrl8", [128, 2, 128], FP8, kind="ExternalInput")
    y_d = nc.dram_tensor("y", [128, I_CHUNKS], F32, kind="ExternalOutput")

    from contextlib import nullcontext

    with tile.TileContext(nc) as tc:
        with (
            tc.tile_pool(name="wp", bufs=1) as wppool,
            tc.tile_pool(name="io", bufs=1) as iopool,
            tc.tile_pool(name="c8", bufs=1) as cpool,
            tc.For_i(0, repeat, 1) if repeat > 1 else nullcontext(),
        ):
            WP = wppool.tile([128, DEPTH, KC, DIM], F32R)
            inpT_sb = iopool.tile([128, KC, NI_SH], F32R, name="inpT_sb")
            dataT_sb = iopool.tile([128, KC, ND_SH], F32R, name="dataT_sb")
            alp = iopool.tile([128, ND_SH], F32, name="alp")
            ones8 = cpool.tile([128, 2, 16], FP8, name="ones8")
            corrl8 = cpool.tile([128, 2, 128], FP8, name="corrl8")
            nc.sync.dma_start(ones8[:], ones_d.ap())
            nc.sync.dma_start(corrl8[:], corrl_d.ap())
            nc.sync.dma_start(alp[:], alphas_d.ap())
            inpT_r = inpT_d.ap().rearrange("(k p) n -> p k n", p=128)
            dataT_r = dataT_d.ap().rearrange("(k p) n -> p k n", p=128)
            for k in range(KC):
                nc.sync.dma_start(inpT_sb[:, k], inpT_r[:, k])
                nc.sync.dma_start(dataT_sb[:, k], dataT_r[:, k])
            nc.sync.dma_start(
                WP[:, 0], gating_d.ap()[0].rearrange("(k p) n -> p k n", p=128)
            )

            # ---- Phase 0: weight-product chains ----
            with (
                tc.tile_pool(name="gv", bufs=1) as gvpool,
                tc.tile_pool(name="vv", bufs=2) as vpool,
                tc.tile_pool(name="p0", bufs=2, space="PSUM") as p0,
            ):
                g_all = gvpool.tile([128, DEPTH, KC, DIM], F32R)
                for l in range(1, DEPTH):
                    nc.sync.dma_start(
                        g_all[:, l],
                        gating_d.ap()[l].rearrange("(k p) n -> p k n", p=128),
                    )
                v_prev = vpool.tile([128, KC, DIM], F32R, tag="v")
                nc.sync.dma_start(
                    v_prev[:], g0T_d.ap().rearrange("(k p) n -> p k n", p=128)
                )
                for l in range(1, DEPTH):
                    # WP_l = WP_{l-1} @ g_l  (lhsT = V_{l-1})
                    for ma in range(KC):
                        ps = p0.tile([128, 512], F32, tag="p0")
                        for k in range(KC):
                            nc.tensor.matmul(
                                ps[:],
                                v_prev[:, k, ma * 128 : (ma + 1) * 128],
                                g_all[:, l, k, :],
                                start=(k == 0),
                                stop=(k == KC - 1),
                            )
                        nc.vector.tensor_copy(WP[:, l, ma, :], ps[:])
                    if l < DEPTH - 1:
                        v_next = vpool.tile([128, KC, DIM], F32R, tag="v", name=f"v{l}")
                        for mw in range(KC):
                            ps = p0.tile([128, 512], F32, tag="p0")
                            for k in range(KC):
                                nc.tensor.matmul(
                                    ps[:],
                                    g_all[:, l, k, mw * 128 : (mw + 1) * 128],
                                    v_prev[:, k, :],
                                    start=(k == 0),
                                    stop=(k == KC - 1),
                                )
                            nc.vector.tensor_copy(v_next[:, mw, :], ps[:])
                        v_prev = v_next

            # ---- Phase A: all gates (i and d side), u-sums ----
            with (
                tc.tile_pool(name="sig", bufs=1) as sigpool,
                tc.tile_pool(name="kb", bufs=2) as kpool,
                tc.tile_pool(name="scr", bufs=2) as scrpool,
                tc.tile_pool(name="gps", bufs=3, space="PSUM") as gpsum,
                tc.tile_pool(name="uip", bufs=1, space="PSUM") as uipsum,
                tc.tile_pool(name="udp", bufs=1, space="PSUM") as udpsum,
                tc.tile_pool(name="kps", bufs=3, space="PSUM") as kpsum,
            ):
                t8i = sigpool.tile([128, DEPTH, KC, NI_SH], FP8)
                t8d = sigpool.tile([128, DEPTH, KC, ND_SH], FP8)
                corr8 = sigpool.tile([128, 2, DEPTH, ND_SH], FP8)
                uw = sigpool.tile([128, I_CHUNKS, DEPTH], F32)
                y_acc = sigpool.tile([128, I_CHUNKS], F32)
                nc.gpsimd.memset(corr8[:], 0.0)
                nc.gpsimd.memset(y_acc[:], 0.0)

                for l in range(DEPTH):
                    for m in range(KC):
                        for nb in range(NI_SH // 512):
                            sl = slice(nb * 512, (nb + 1) * 512)
                            ps = gpsum.tile([128, 512], F32, tag="gps")
                            for k in range(KC):
                                nc.tensor.matmul(
                                    ps[:],
                                    WP[:, l, k, m * 128 : (m + 1) * 128],
                                    inpT_sb[:, k, sl],
                                    start=(k == 0),
                                    stop=(k == KC - 1),
                                )
                            nc.scalar.activation(
                                t8i[:, l, m, sl], ps[:], AFT.Tanh, scale=2.0
                            )
                    for m in range(KC):
                        for nb in range(ND_SH // 512):
                            sl = slice(nb * 512, (nb + 1) * 512)
                            ps = gpsum.tile([128, 512], F32, tag="gps")
                            for k in range(KC):
                                nc.tensor.matmul(
                                    ps[:],
                                    WP[:, l, k, m * 128 : (m + 1) * 128],
                                    dataT_sb[:, k, sl],
                                    start=(k == 0),
                                    stop=(k == KC - 1),
                                )
                            nc.scalar.activation(
                                t8d[:, l, m, sl], ps[:], AFT.Tanh, scale=2.0
                            )
                    # u_i for this layer -> uw[:, ic, l] = sum + 512
                    for ic in range(I_CHUNKS):
                        isl = slice(ic * 128, (ic + 1) * 128)
                        uips = uipsum.tile([128, 16], F32, tag="uip")
                        for mm in (0, 2):
                            nc.tensor.matmul(
                                uips[:],
                                t8i[:, l, mm : mm + 2, isl],
                                ones8[:],
                                start=(mm == 0),
                                stop=(mm == 2),
                                perf_mode=DR,
                            )
                        nc.vector.tensor_scalar_add(
                            uw[:, ic, l : l + 1], uips[:, 0:1], 512.0
                        )
                    # u_d for this layer -> corr8[0, 0, l, :] = fp8(u_d/64)
                    for ds in range(DSL):
                        dsl = slice(ds * 512, (ds + 1) * 512)
                        udps = udpsum.tile([16, 512], F32, tag="udp")
                        for mm in (0, 2):
                            nc.tensor.matmul(
                                udps[:],
                                ones8[:],
                                t8d[:, l, mm : mm + 2, dsl],
                                start=(mm == 0),
                                stop=(mm == 2),
                                perf_mode=DR,
                            )
                        nc.vector.tensor_scalar_mul(
                            corr8[0:1, 0, l, dsl], udps[0:1, :], 1.0 / 64
                        )

                # ---- Phase B: fp8 K-product ----
                for ic in range(I_CHUNKS):
                    isl = slice(ic * 128, (ic + 1) * 128)
                    for ds in range(DSL):
                        dsl = slice(ds * 512, (ds + 1) * 512)
                        kb = kpool.tile([128, 512], F32, tag="kb")
                        for l in range(DEPTH):
                            kps = kpsum.tile([128, 512], F32, tag="kps")
                            for mm in (0, 2):
                                nc.tensor.matmul(
                                    kps[:],
                                    t8i[:, l, mm : mm + 2, isl],
                                    t8d[:, l, mm : mm + 2, dsl],
                                    start=(mm == 0),
                                    stop=False,
                                    perf_mode=DR,
                                )
                            nc.tensor.matmul(
                                kps[:],
                                corrl8[:],
                                corr8[:, :, l, dsl],
                                start=False,
                                stop=True,
                                perf_mode=DR,
                            )
                            if l == 0:
                                nc.vector.scalar_tensor_tensor(
                                    kb[:], kps[:], uw[:, ic, 0:1], alp[:, dsl],
                                    ADD, MULT,
                                )
                            elif l < DEPTH - 1:
                                nc.vector.scalar_tensor_tensor(
                                    kb[:], kps[:], uw[:, ic, l : l + 1], kb[:],
                                    ADD, MULT,
                                )
                            else:
                                part = scrpool.tile([128, 1], F32, tag="part")
                                nc.vector.scalar_tensor_tensor(
                                    kb[:], kps[:], uw[:, ic, 3:4], kb[:],
                                    ADD, MULT, accum_out=part[:, 0:1],
                                )
                                nc.vector.tensor_add(
                                    y_acc[:, ic : ic + 1],
                                    y_acc[:, ic : ic + 1],
                                    part[:, 0:1],
                                )
                nc.vector.tensor_scalar_mul(y_acc[:], y_acc[:], 2.0 ** -35)
                nc.sync.dma_start(y_d.ap(), y_acc[:])

    nc.compile()
    return nc


def _get_nc():
    global _NC
    if _NC is None:
        _NC = _build_fp8()
    return _NC


def _make_in_maps(inp, data, gating, alphas):
    e4 = ml_dtypes.float8_e4m3fn
    ones8 = np.ones((128, 2, 16), dtype=e4)
    corrl8 = np.zeros((128, 2, 128), dtype=e4)
    corrl8[0, 0, :] = 64.0
    g0T = np.ascontiguousarray(gating[0].T)
    in_maps = []
    for r in range(R):
        inpT = np.ascontiguousarray(inp[r * NI_SH : (r + 1) * NI_SH].T)
        for c in range(C):
            dataT = np.ascontiguousarray(data[c * ND_SH : (c + 1) * ND_SH].T)
            al = np.ascontiguousarray(
                np.broadcast_to(alphas[c * ND_SH : (c + 1) * ND_SH], (128, ND_SH))
            )
            in_maps.append(
                {
                    "inpT": inpT,
                    "dataT": dataT,
                    "gating": gating,
                    "g0T": g0T,
                    "alphas_b": al,
                    "ones8": ones8,
                    "corrl8": corrl8,
                }
            )
    return in_maps


def kernel(inp, data, gating, alphas):
    inp = np.ascontiguousarray(np.asarray(inp, dtype=np.float32))
    data = np.ascontiguousarray(np.asarray(data, dtype=np.float32))
    gating = np.ascontiguousarray(np.asarray(gating, dtype=np.float32))
    alphas = np.ascontiguousarray(np.asarray(alphas, dtype=np.float32))

    nc = _get_nc()
    in_maps = _make_in_maps(inp, data, gating, alphas)
    res = run_bass_kernel_spmd(nc, in_maps, core_ids=list(range(R * C))).results

    y = np.empty(N_I, dtype=np.float32)
    for r in range(R):
        acc = res[r * C]["y"].T.reshape(NI_SH).copy()
        for c in range(1, C):
            acc += res[r * C + c]["y"].T.reshape(NI_SH)
        y[r * NI_SH : (r + 1) * NI_SH] = acc
    return y


# revision 4
# speedup vs baseline: 1.1029x; 1.0018x over previous
"""Trainium2 Bass kernel for the DLGN kernel-machine problem (fp8 DoubleRow).

Reference (fp32):
    ig = inp @ g0; dg = data @ g0
    K  = sig(4 ig) @ sig(4 dg).T
    for l in 1..3: ig @= g_l; dg @= g_l; K *= (sig(4 ig) @ sig(4 dg).T)/512
    out = K @ alphas                                  # [4096]

Design (8 cores, R=2 x C=4: inp rows split in 2, data rows in 4; each core
computes y_partial[r-block] over its data block; host sums C partials):
  - Gate chains are LINEAR in the preactivations: ig_l = inp @ (g0 g1..g_l).
    The kernel precomputes both weight-product chains on device (WP_l and
    its transpose chain V_l, plain f32r matmuls) and computes every layer's
    preactivation directly from inp/data -- no serial layer dependency, no
    PSUM->SBUF preactivation copies.
  - K-product runs in fp8 (float8e4) DoubleRow mode at 0.5 cyc/row (4x the
    f32r rate). Precision is recovered by tanh-centering: store
    t = fp8(tanh(2x)) (= 2 sig(4x) - 1), then
      sig-gram G = (T + u_i + u_d + 512) / 4,  T = t_i^T t_d (fp8 gram),
    with u_* = exact column sums of t (ones-matmuls). u_i enters as a
    per-partition scalar in the vector-engine multiply; u_d via one extra
    zero-padded DoubleRow pair (row0 = 64 * fp8(u_d/64)). The constant
    1/(4*2048^3) is applied once to y at the end (2^-35). Measured rel err
    ~1e-2 vs the 2e-2 gate (CPU model 9.8e-3).
  - ACT engine runs ONLY Tanh (scale=2, direct fp8 output -- bit-exact vs
    RNE cast); running K-product stays on DVE scalar_tensor_tensor with the
    alphas folded into layer 0 and accum_out row-sum on layer 3.
"""

import numpy as np
import ml_dtypes

import concourse.tile as tile
from concourse import bacc, mybir
from concourse.bass_utils import run_bass_kernel_spmd

WIDTH = 512
DEPTH = 4
DIM = 512
N_I = 4096
N_D = 8192
R, C = 2, 4
NI_SH = N_I // R  # 2048
ND_SH = N_D // C  # 2048
I_CHUNKS = NI_SH // 128  # 16
DSL = ND_SH // 512  # 4 d-blocks of 512
KC = DIM // 128  # 4

F32 = mybir.dt.float32
F32R = mybir.dt.float32r
FP8 = mybir.dt.float8e4
AFT = mybir.ActivationFunctionType
MULT = mybir.AluOpType.mult
ADD = mybir.AluOpType.add
DR = mybir.MatmulPerfMode.DoubleRow

_NC = None


def _build_fp8(repeat=1):
    nc = bacc.Bacc("TRN2", target_bir_lowering=False, debug=False, num_devices=8)

    inpT_d = nc.dram_tensor("inpT", [DIM, NI_SH], F32R, kind="ExternalInput")
    dataT_d = nc.dram_tensor("dataT", [DIM, ND_SH], F32R, kind="ExternalInput")
    gating_d = nc.dram_tensor("gating", [DEPTH, DIM, DIM], F32R, kind="ExternalInput")
    g0T_d = nc.dram_tensor("g0T", [DIM, DIM], F32R, kind="ExternalInput")
    alphas_d = nc.dram_tensor("alphas_b", [128, ND_SH], F32, kind="ExternalInput")
    ones_d = nc.dram_tensor("ones8", [128, 2, 16], FP8, kind="ExternalInput")
    corrl_d = nc.dram_tensor("corrl8", [128, 2, 128], FP8, kind="ExternalInput")
    y_d = nc.dram_tensor("y", [128, I_CHUNKS], F32, kind="ExternalOutput")

    from contextlib import nullcontext

    with tile.TileContext(nc) as tc:
        with (
            tc.tile_pool(name="wp", bufs=1) as wppool,
            tc.tile_pool(name="io", bufs=1) as iopool,
            tc.tile_pool(name="c8", bufs=1) as cpool,
            tc.For_i(0, repeat, 1) if repeat > 1 else nullcontext(),
        ):
            WP = wppool.tile([128, DEPTH, KC, DIM], F32R)
            inpT_sb = iopool.tile([128, KC, NI_SH], F32R, name="inpT_sb")
            dataT_sb = iopool.tile([128, KC, ND_SH], F32R, name="dataT_sb")
            alp = iopool.tile([128, ND_SH], F32, name="alp")
            ones8 = cpool.tile([128, 2, 16], FP8, name="ones8")
            corrl8 = cpool.tile([128, 2, 128], FP8, name="corrl8")
            nc.sync.dma_start(
                WP[:, 0], gating_d.ap()[0].rearrange("(k p) n -> p k n", p=128)
            )
            nc.sync.dma_start(ones8[:], ones_d.ap())
            nc.sync.dma_start(corrl8[:], corrl_d.ap())
            nc.sync.dma_start(alp[:], alphas_d.ap())
            inpT_r = inpT_d.ap().rearrange("(k p) n -> p k n", p=128)
            dataT_r = dataT_d.ap().rearrange("(k p) n -> p k n", p=128)
            for k in range(KC):
                nc.scalar.dma_start(inpT_sb[:, k], inpT_r[:, k])
                nc.scalar.dma_start(dataT_sb[:, k], dataT_r[:, k])

            # ---- Phase 0: weight-product chains ----
            with (
                tc.tile_pool(name="gv", bufs=1) as gvpool,
                tc.tile_pool(name="vv", bufs=2) as vpool,
                tc.tile_pool(name="p0", bufs=2, space="PSUM") as p0,
            ):
                g_all = gvpool.tile([128, DEPTH, KC, DIM], F32R)
                for l in range(1, DEPTH):
                    nc.sync.dma_start(
                        g_all[:, l],
                        gating_d.ap()[l].rearrange("(k p) n -> p k n", p=128),
                    )
                v_prev = vpool.tile([128, KC, DIM], F32R, tag="v")
                nc.sync.dma_start(
                    v_prev[:], g0T_d.ap().rearrange("(k p) n -> p k n", p=128)
                )
                for l in range(1, DEPTH):
                    # WP_l = WP_{l-1} @ g_l  (lhsT = V_{l-1})
                    for ma in range(KC):
                        ps = p0.tile([128, 512], F32, tag="p0")
                        for k in range(KC):
                            nc.tensor.matmul(
                                ps[:],
                                v_prev[:, k, ma * 128 : (ma + 1) * 128],
                                g_all[:, l, k, :],
                                start=(k == 0),
                                stop=(k == KC - 1),
                            )
                        nc.vector.tensor_copy(WP[:, l, ma, :], ps[:])
                    if l < DEPTH - 1:
                        v_next = vpool.tile([128, KC, DIM], F32R, tag="v", name=f"v{l}")
                        for mw in range(KC):
                            ps = p0.tile([128, 512], F32, tag="p0")
                            for k in range(KC):
                                nc.tensor.matmul(
                                    ps[:],
                                    g_all[:, l, k, mw * 128 : (mw + 1) * 128],
                                    v_prev[:, k, :],
                                    start=(k == 0),
                                    stop=(k == KC - 1),
                                )
                            nc.vector.tensor_copy(v_next[:, mw, :], ps[:])
                        v_prev = v_next

            # ---- Phase A: all gates (i and d side), u-sums ----
            with (
                tc.tile_pool(name="sig", bufs=1) as sigpool,
                tc.tile_pool(name="kb", bufs=4) as kpool,
                tc.tile_pool(name="scr", bufs=4) as scrpool,
                tc.tile_pool(name="gps", bufs=3, space="PSUM") as gpsum,
                tc.tile_pool(name="uip", bufs=1, space="PSUM") as uipsum,
                tc.tile_pool(name="udp", bufs=1, space="PSUM") as udpsum,
                tc.tile_pool(name="kps", bufs=3, space="PSUM") as kpsum,
            ):
                t8i = sigpool.tile([128, DEPTH, KC, NI_SH], FP8)
                t8d = sigpool.tile([128, DEPTH, KC, ND_SH], FP8)
                corr8 = sigpool.tile([128, 2, DEPTH, ND_SH], FP8)
                uw = sigpool.tile([128, I_CHUNKS, DEPTH], F32)
                y_acc = sigpool.tile([128, I_CHUNKS], F32)
                nc.gpsimd.memset(corr8[:], 0.0)
                nc.gpsimd.memset(y_acc[:], 0.0)

                for l in range(DEPTH):
                    for m in range(KC):
                        for nb in range(NI_SH // 512):
                            sl = slice(nb * 512, (nb + 1) * 512)
                            ps = gpsum.tile([128, 512], F32, tag="gps")
                            for k in range(KC):
                                nc.tensor.matmul(
                                    ps[:],
                                    WP[:, l, k, m * 128 : (m + 1) * 128],
                                    inpT_sb[:, k, sl],
                                    start=(k == 0),
                                    stop=(k == KC - 1),
                                )
                            nc.scalar.activation(
                                t8i[:, l, m, sl], ps[:], AFT.Tanh, scale=2.0
                            )
                    for m in range(KC):
                        for nb in range(ND_SH // 512):
                            sl = slice(nb * 512, (nb + 1) * 512)
                            ps = gpsum.tile([128, 512], F32, tag="gps")
                            for k in range(KC):
                                nc.tensor.matmul(
                                    ps[:],
                                    WP[:, l, k, m * 128 : (m + 1) * 128],
                                    dataT_sb[:, k, sl],
                                    start=(k == 0),
                                    stop=(k == KC - 1),
                                )
                            nc.scalar.activation(
                                t8d[:, l, m, sl], ps[:], AFT.Tanh, scale=2.0
                            )
                    # u_i for this layer -> uw[:, ic, l] = sum + 512
                    for ic in range(I_CHUNKS):
                        isl = slice(ic * 128, (ic + 1) * 128)
                        uips = uipsum.tile([128, 16], F32, tag="uip")
                        for mm in (0, 2):
                            nc.tensor.matmul(
                                uips[:],
                                t8i[:, l, mm : mm + 2, isl],
                                ones8[:],
                                start=(mm == 0),
                                stop=(mm == 2),
                                perf_mode=DR,
                            )
                        nc.vector.tensor_scalar_add(
                            uw[:, ic, l : l + 1], uips[:, 0:1], 512.0
                        )
                    # u_d for this layer -> corr8[0, 0, l, :] = fp8(u_d/64)
                    for ds in range(DSL):
                        dsl = slice(ds * 512, (ds + 1) * 512)
                        udps = udpsum.tile([16, 512], F32, tag="udp")
                        for mm in (0, 2):
                            nc.tensor.matmul(
                                udps[:],
                                ones8[:],
                                t8d[:, l, mm : mm + 2, dsl],
                                start=(mm == 0),
                                stop=(mm == 2),
                                perf_mode=DR,
                            )
                        nc.vector.tensor_scalar_mul(
                            corr8[0:1, 0, l, dsl], udps[0:1, :], 1.0 / 64
                        )

                # ---- Phase B: fp8 K-product ----
                for ic in range(I_CHUNKS):
                    isl = slice(ic * 128, (ic + 1) * 128)
                    for ds in range(DSL):
                        dsl = slice(ds * 512, (ds + 1) * 512)
                        kb = kpool.tile([128, 512], F32, tag="kb")
                        for l in range(DEPTH):
                            kps = kpsum.tile([128, 512], F32, tag="kps")
                            for mm in (0, 2):
                                nc.tensor.matmul(
                                    kps[:],
                                    t8i[:, l, mm : mm + 2, isl],
                                    t8d[:, l, mm : mm + 2, dsl],
                                    start=(mm == 0),
                                    stop=False,
                                    perf_mode=DR,
                                )
                            nc.tensor.matmul(
                                kps[:],
                                corrl8[:],
                                corr8[:, :, l, dsl],
                                start=False,
                                stop=True,
                                perf_mode=DR,
                            )
                            if l == 0:
                                nc.vector.scalar_tensor_tensor(
                                    kb[:], kps[:], uw[:, ic, 0:1], alp[:, dsl],
                                    ADD, MULT,
                                )
                            elif l == 1:
                                nc.vector.scalar_tensor_tensor(
                                    kb[:], kps[:], uw[:, ic, l : l + 1], kb[:],
                                    ADD, MULT,
                                )
                            elif l == 2:
                                nc.vector.scalar_tensor_tensor(
                                    kb[:], kps[:], uw[:, ic, l : l + 1], kb[:],
                                    ADD, MULT,
                                )
                            else:
                                part = scrpool.tile([128, 1], F32, tag="part")
                                nc.vector.scalar_tensor_tensor(
                                    kb[:], kps[:], uw[:, ic, 3:4], kb[:],
                                    ADD, MULT, accum_out=part[:, 0:1],
                                )
                                nc.gpsimd.tensor_add(
                                    y_acc[:, ic : ic + 1],
                                    y_acc[:, ic : ic + 1],
                                    part[:, 0:1],
                                )
                nc.vector.tensor_scalar_mul(y_acc[:], y_acc[:], 2.0 ** -35)
                nc.sync.dma_start(y_d.ap(), y_acc[:])

    nc.compile()
    return nc


def _get_nc():
    global _NC
    if _NC is None:
        _NC = _build_fp8()
    return _NC


def _make_in_maps(inp, data, gating, alphas):
    e4 = ml_dtypes.float8_e4m3fn
    ones8 = np.ones((128, 2, 16), dtype=e4)
    corrl8 = np.zeros((128, 2, 128), dtype=e4)
    corrl8[0, 0, :] = 64.0
    g0T = np.ascontiguousarray(gating[0].T)
    in_maps = []
    for r in range(R):
        inpT = np.ascontiguousarray(inp[r * NI_SH : (r + 1) * NI_SH].T)
        for c in range(C):
            dataT = np.ascontiguousarray(data[c * ND_SH : (c + 1) * ND_SH].T)
            al = np.ascontiguousarray(
                np.broadcast_to(alphas[c * ND_SH : (c + 1) * ND_SH], (128, ND_SH))
            )
            in_maps.append(
                {
                    "inpT": inpT,
                    "dataT": dataT,
                    "gating": gating,
                    "g0T": g0T,
                    "alphas_b": al,
                    "ones8": ones8,
                    "corrl8": corrl8,
                }
            )
    return in_maps


def kernel(inp, data, gating, alphas):
    inp = np.ascontiguousarray(np.asarray(inp, dtype=np.float32))
    data = np.ascontiguousarray(np.asarray(data, dtype=np.float32))
    gating = np.ascontiguousarray(np.asarray(gating, dtype=np.float32))
    alphas = np.ascontiguousarray(np.asarray(alphas, dtype=np.float32))

    nc = _get_nc()
    in_maps = _make_in_maps(inp, data, gating, alphas)
    res = run_bass_kernel_spmd(nc, in_maps, core_ids=list(range(R * C))).results

    y = np.empty(N_I, dtype=np.float32)
    for r in range(R):
        acc = res[r * C]["y"].T.reshape(NI_SH).copy()
        for c in range(1, C):
            acc += res[r * C + c]["y"].T.reshape(NI_SH)
        y[r * NI_SH : (r + 1) * NI_SH] = acc
    return y
